# revision 22
# baseline (speedup 1.0000x reference)
"""Trainium2 Bass kernel for nn_CrossAttention (b=2, s1=2048, s2=1024, H=16, hd=64).

Sharding: 8 cores = 2 batches x 4 head-groups (4 heads each).

v2 design (vs v1 baseline at ~266us):
  - Dense upfront preamble keeps PE p-state at max: kproj -> LN-k -> qproj ->
    vproj -> LN-q -> RoPE, then an ACT-paced attention phase, then out-proj.
  - LN stats batched: per-slice [4,512] stat matmuls land at PSUM partition
    bases 0/32/64/96 via col tile_position, so the mean/var chain and the
    single ACT Rsqrt cover all slices in one [100,512] pass.
  - LN affine folded into the broadcast matmul stationary (selector rows
    pre-multiplied by qn_w/kn_w on host); apply = 2 DVE ops reading PSUM.
  - Scores for the head pair of each 128-channel chunk are emitted
    back-to-back with K=64 row tiles (0,0)/(64,0) -> concurrent on the PE.
  - Softmax denominator rides as a ones-column of v (M=65 AV matmuls).
  - Per-unit (pair x s1-half) normalization via DMA-replicated reciprocal
    rows (no PSUM, no PE).
  - Output written bf16 (host accumulates partials in fp32).
"""

import numpy as np

B, S1, S2, CIN, H, HD = 2, 2048, 1024, 1024, 16, 64
HPC = 4                # heads per core
CPC = HPC * HD         # 256 channels per core
P = 128
KC = CIN // P          # 8 cin chunks
MC = CPC // P          # 2 channel chunks (= head pairs)
NQ = S1 // 512         # 4 s1 slices
NK = S2 // 512         # 2 s2 slices
M2 = S2 // P           # 8 s2 chunks
SCALE = HD ** -0.5
EPS = 1e-6

_NC_CACHE = {}


def _legalize_waits(nc, mybir, limit=1):
    """Split instructions carrying >limit semaphore waits into a chain of
    single-wait NOPs on the same engine followed by the instruction."""
    n_split = 0
    for fn in nc.m.functions:
        for bb in fn.blocks:
            out = []
            for inst in bb.instructions:
                si = inst.sync_info
                waits = list(si.on_wait) if si is not None and si.on_wait else []
                if len(waits) > limit:
                    for i, w in enumerate(waits[:-limit]):
                        nop = mybir.InstNoOp(
                            name=f"{inst.name}-lw{i}", ins=[], outs=[])
                        nop.engine = inst.engine
                        nop.sync_info = mybir.SyncInfo(on_wait=[w], on_update=[])
                        try:
                            nc.register_instruction(nop, overwrite=True)
                        except Exception:
                            pass
                        out.append(nop)
                    inst.sync_info = mybir.SyncInfo(
                        on_wait=waits[-limit:], on_update=list(si.on_update))
                    n_split += 1
                out.append(inst)
            bb.instructions = out
    return n_split


def _build_nc():
    from contextlib import ExitStack

    import concourse.bass as bass
    import concourse.mybir as mybir
    import concourse.tile as tile

    f32 = mybir.dt.float32
    bf16 = mybir.dt.bfloat16
    AF = mybir.ActivationFunctionType
    OP = mybir.AluOpType

    nc = bass.Bass()

    def din(name, shape, dt=bf16):
        return nc.dram_tensor(name, list(shape), dt, kind="ExternalInput")

    xT = din("xT", (CIN, S1))
    yTc = din("yTc", (P, KC, S2))
    qwc = din("qwc", (P, KC, CPC))
    kwc = din("kwc", (P, KC, CPC))
    vwc = din("vwc", (P, KC, CPC))
    owc = din("owc", (P, MC, CIN))
    qb = din("qb", (P, MC), f32)
    kb = din("kb", (P, MC), f32)
    vb = din("vb", (CPC,), f32)
    qnb = din("qnb", (P, MC), f32)
    knb = din("knb", (P, MC), f32)
    rwq = din("rwq", (100, CPC))
    rwk = din("rwk", (36, CPC))
    seld = din("sel", (P, MC, HPC))
    cosf = din("cosf", (P, S1))
    sinf = din("sinf", (P, S1))
    outT = nc.dram_tensor("outT", [CIN, S1], bf16, kind="ExternalOutput")

    with tile.TileContext(nc) as tc, ExitStack() as ctx:
        consts = ctx.enter_context(tc.tile_pool(name="consts", bufs=1))
        pers = ctx.enter_context(tc.tile_pool(name="pers", bufs=1))
        xs = ctx.enter_context(tc.tile_pool(name="xs", bufs=4))
        tmp = ctx.enter_context(tc.tile_pool(name="tmp", bufs=2))
        rop = ctx.enter_context(tc.tile_pool(name="rop", bufs=2))
        att = ctx.enter_context(tc.tile_pool(name="att", bufs=2))
        expp = ctx.enter_context(tc.tile_pool(name="expp", bufs=6))
        osbp = ctx.enter_context(tc.tile_pool(name="osbp", bufs=4))

        ctxA = ctx.enter_context(ExitStack())
        pp = ctxA.enter_context(tc.tile_pool(name="pp", bufs=4, space="PSUM"))
        pst = ctxA.enter_context(tc.tile_pool(name="pst", bufs=2, space="PSUM"))
        pvv = ctxA.enter_context(tc.tile_pool(name="pvv", bufs=2, space="PSUM"))

        # ---- constants ----
        # k-proj inputs first (scalar ring), split per k-chunk for streaming
        kwt_sb = consts.tile([P, KC, CPC], bf16)
        yT_sb = consts.tile([P, KC, S2], bf16)
        for k in range(KC):
            nc.scalar.dma_start(out=kwt_sb[:, k, :], in_=kwc[:, k, :])
            nc.scalar.dma_start(out=yT_sb[:, k, :], in_=yTc[:, k, :])
        # q-proj weights on sync ring (x stream follows there)
        qwt_sb = consts.tile([P, KC, CPC], bf16)
        for k in range(KC):
            nc.sync.dma_start(out=qwt_sb[:, k, :], in_=qwc[:, k, :])
        # the rest on gpsimd/vector rings, ordered by first use
        sel_sb = consts.tile([P, MC, HPC], bf16)
        nc.gpsimd.dma_start(out=sel_sb, in_=seld[:])
        rwk_sb = consts.tile([36, CPC], bf16)
        nc.gpsimd.dma_start(out=rwk_sb, in_=rwk[:])
        rwq_sb = consts.tile([100, CPC], bf16)
        nc.gpsimd.dma_start(out=rwq_sb, in_=rwq[:])
        kb_sb = consts.tile([P, MC], f32)
        nc.gpsimd.dma_start(out=kb_sb, in_=kb[:])
        qb_sb = consts.tile([P, MC], f32)
        nc.gpsimd.dma_start(out=qb_sb, in_=qb[:])
        knb_sb = consts.tile([P, MC], f32)
        nc.gpsimd.dma_start(out=knb_sb, in_=knb[:])
        qnb_sb = consts.tile([P, MC], f32)
        nc.gpsimd.dma_start(out=qnb_sb, in_=qnb[:])
        vwt_sb = consts.tile([P, KC, CPC], bf16)
        for k in range(KC):
            nc.scalar.dma_start(out=vwt_sb[:, k, :], in_=vwc[:, k, :])
        cosf_sb = consts.tile([P, S1], bf16)
        nc.scalar.dma_start(out=cosf_sb, in_=cosf[:])
        sinf_sb = consts.tile([P, S1], bf16)
        nc.scalar.dma_start(out=sinf_sb, in_=sinf[:])
        owt_sb = consts.tile([P, MC, CIN], bf16)
        nc.scalar.dma_start(out=owt_sb, in_=owc[:])
        vbb_sb = consts.tile([P, CPC], f32)
        vb_ap = vb[:]
        nc.gpsimd.dma_start(
            out=vbb_sb,
            in_=bass.AP(tensor=vb_ap.tensor, offset=vb_ap.offset,
                        ap=[[0, P]] + list(vb_ap.ap)),
        )
        eps_t = consts.tile([100, 1], f32)
        nc.vector.memset(eps_t, EPS)
        dm = consts.tile([1, 8], f32)
        nc.vector.memset(dm, 1.0)
        dmo = consts.tile([1, 8], f32)
        # warm the ACT sqrt table set early (hidden under DMA warmup)
        nc.scalar.activation(out=dmo[:], in_=dm[:], func=AF.Sqrt)

        # ---- persistent activations ----
        qT_sb = pers.tile([P, MC, S1], bf16)
        kT_sb = pers.tile([P, MC, S2], bf16)
        # v column 64 is all-ones: the AV matmul emits the softmax denominator
        # as PSUM row 64 for free (M=65 is paced by N, same as M=64).
        v_sb = pers.tile([P, M2, HPC, HD + 1], bf16)
        onorm = pers.tile([P, MC, S1], bf16)
        nc.vector.memset(v_sb[:, :, :, HD:HD + 1], 1.0)
        ones64 = consts.tile([1, HD], bf16)
        nc.vector.memset(ones64, 1.0)

        # ---- layernorm helper (batched stats at 32-partition bases) ----
        def layernorm(src, NS, rw_sb, b_sb, pfx):
            """src: [P, MC, NS*512]; stats for slice n at partition base 32n."""
            nparts = 32 * (NS - 1) + 4
            pss = pst.tile([nparts, 512], f32, name=f"{pfx}pss", tag="st")
            psq = pst.tile([nparts, 512], f32, name=f"{pfx}psq", tag="st")
            nc.vector.memset(pss, 0.0)
            nc.vector.memset(psq, 0.0)
            sqs = []
            for n in range(NS):
                sl = slice(n * 512, (n + 1) * 512)
                for c in range(MC):
                    sq = tmp.tile([P, 512], bf16, name=f"{pfx}sq{c}_{n}",
                                  tag="sq", bufs=4)
                    nc.vector.tensor_mul(sq[:], src[:, c, sl], src[:, c, sl])
                    sqs.append(sq)
            for c in range(MC):
                st = (c == 0)
                sp = (c == MC - 1)
                for n in range(NS):
                    sl = slice(n * 512, (n + 1) * 512)
                    r0 = 32 * n
                    nc.tensor.matmul(pss[r0:r0 + 4, :], sel_sb[:, c, :],
                                     src[:, c, sl], start=st, stop=sp,
                                     tile_position=(0, r0),
                                     skip_group_check=True)
                    nc.tensor.matmul(psq[r0:r0 + 4, :], sel_sb[:, c, :],
                                     sqs[n * MC + c][:], start=st, stop=sp,
                                     tile_position=(0, r0),
                                     skip_group_check=True)
            # mean/var chain over the whole batched span (junk rows harmless)
            mu = tmp.tile([nparts, 512], f32, name=f"{pfx}mu", tag="mu")
            nc.vector.tensor_copy(mu[:], pss[:])
            musq = tmp.tile([nparts, 512], f32, name=f"{pfx}musq", tag="musq")
            nc.vector.tensor_mul(musq[:], mu[:], mu[:])
            var = tmp.tile([nparts, 512], f32, name=f"{pfx}var", tag="var")
            nc.vector.tensor_sub(var[:], psq[:], musq[:])
            sd = tmp.tile([nparts, 512], f32, name=f"{pfx}sd", tag="sd")
            nc.scalar.activation(out=sd[:], in_=var[:], func=AF.Sqrt,
                                 bias=eps_t[0:nparts, :], scale=1.0)
            Af = tmp.tile([nparts, 512], f32, name=f"{pfx}Af", tag="Af")
            nc.vector.reciprocal(Af[:], sd[:])
            Ab = tmp.tile([nparts, 512], bf16, name=f"{pfx}Ab", tag="Ab")
            nc.vector.tensor_copy(Ab[:], Af[:])
            Bb = tmp.tile([nparts, 512], bf16, name=f"{pfx}Bb", tag="Bb")
            nc.vector.scalar_tensor_tensor(
                out=Bb[:], in0=mu[:], scalar=-1.0, in1=Af[:],
                op0=OP.mult, op1=OP.mult)                     # B = -mu*rstd
            # broadcast (w folded into stationary rows) + apply
            for c in range(MC):
                for n in range(NS):
                    r0 = 32 * n
                    sl = slice(n * 512, (n + 1) * 512)
                    psA = pp.tile([P, 512], f32, name=f"{pfx}psA{c}_{n}",
                                  tag="proj")
                    nc.tensor.matmul(
                        psA[:], rw_sb[r0:r0 + 4, c * P:(c + 1) * P],
                        Ab[r0:r0 + 4, :], start=True, stop=True,
                        tile_position=(r0, 0))
                    psB = pp.tile([P, 512], f32, name=f"{pfx}psB{c}_{n}",
                                  tag="proj")
                    nc.tensor.matmul(
                        psB[:], rw_sb[r0:r0 + 4, c * P:(c + 1) * P],
                        Bb[r0:r0 + 4, :], start=True, stop=True,
                        tile_position=(r0, 0))
                    nc.vector.tensor_mul(src[:, c, sl], psA[:],
                                         src[:, c, sl])
                    nc.vector.scalar_tensor_tensor(
                        out=src[:, c, sl], in0=psB[:],
                        scalar=b_sb[:, c:c + 1], in1=src[:, c, sl],
                        op0=OP.add, op1=OP.add)

        # ---- k projection ----
        for n2 in range(NK):
            sl = slice(n2 * 512, (n2 + 1) * 512)
            ps = [pp.tile([P, 512], f32, name=f"psk{c}_{n2}", tag="proj")
                  for c in range(MC)]
            for k in range(KC):
                for c in range(MC):
                    nc.tensor.matmul(
                        ps[c][:], kwt_sb[:, k, c * P:(c + 1) * P],
                        yT_sb[:, k, sl], start=(k == 0), stop=(k == KC - 1))
            for c in range(MC):
                nc.vector.tensor_scalar_add(kT_sb[:, c, sl], ps[c][:],
                                            kb_sb[:, c:c + 1])

        # ---- q projection (x streamed per k-chunk, 1024-col halves) ----
        def qproj_half(half):
            hsl = slice(half * 1024, (half + 1) * 1024)
            ps = [[pp.tile([P, 512], f32, name=f"psq{c}_{half}{n}", tag="proj")
                   for n in range(2)] for c in range(MC)]
            for k in range(KC):
                xt = xs.tile([P, 1024], bf16, name=f"xt{half}_{k}", tag="xs")
                nc.sync.dma_start(out=xt, in_=xT[k * P:(k + 1) * P, hsl])
                for c in range(MC):
                    for n in range(2):
                        nc.tensor.matmul(
                            ps[c][n][:], qwt_sb[:, k, c * P:(c + 1) * P],
                            xt[:, n * 512:(n + 1) * 512],
                            start=(k == 0), stop=(k == KC - 1))
            for c in range(MC):
                for n in range(2):
                    sl = slice(half * 1024 + n * 512, half * 1024 + (n + 1) * 512)
                    nc.vector.tensor_scalar_add(qT_sb[:, c, sl], ps[c][n][:],
                                                qb_sb[:, c:c + 1])

        # ---- v projection chunk: v[s2-part, channel] ----
        def vproj(m):
            psv = pvv.tile([P, CPC], f32, name=f"psv{m}", tag="pv")
            for k in range(KC):
                nc.tensor.matmul(
                    psv[:], yT_sb[:, k, m * P:(m + 1) * P], vwt_sb[:, k, :],
                    start=(k == 0), stop=(k == KC - 1))
            nc.vector.tensor_add(
                v_sb[:, m, :, 0:HD],
                psv.rearrange("p (h d) -> p h d", h=HPC),
                vbb_sb.rearrange("p (h d) -> p h d", h=HPC))

        # fill pool-ring boundary bubbles: alternate independent PE work
        qproj_half(0)
        for m in range(4):
            vproj(m)
        layernorm(kT_sb, NK, rwk_sb, knb_sb, "k")
        qproj_half(1)
        for m in range(4, M2):
            vproj(m)
        layernorm(qT_sb, NQ, rwq_sb, qnb_sb, "q")

        # warm the ACT exp table set (hidden under RoPE / attention warmup)
        nc.scalar.activation(out=dmo[:], in_=dm[:], func=AF.Exp)

        # ---- RoPE on q, in place (after LN, like the reference) ----
        for c in range(MC):
            for half in range(2):
                hsl = slice(half * 1024, (half + 1) * 1024)
                qsw = rop.tile([P, 1024], bf16, name=f"qsw{c}_{half}",
                               tag="qsw")
                for blk in range(4):
                    d_src = (blk ^ 1) * 32
                    nc.scalar.dma_start(
                        out=qsw[blk * 32:(blk + 1) * 32, :],
                        in_=qT_sb[d_src:d_src + 32, c, hsl])
                t = rop.tile([P, 1024], bf16, name=f"rt{c}_{half}", tag="rt")
                nc.vector.tensor_mul(t[:], qsw[:], sinf_sb[:, hsl])
                nc.vector.tensor_mul(qT_sb[:, c, hsl], qT_sb[:, c, hsl],
                                     cosf_sb[:, hsl])
                nc.vector.tensor_add(qT_sb[:, c, hsl], qT_sb[:, c, hsl], t[:])

        # ---- attention: units = (pair c, s1-half), ACT-paced ----
        ctxA.close()
        ctxB = ctx.enter_context(ExitStack())
        pso = ctxB.enter_context(tc.tile_pool(name="pso", bufs=2, space="PSUM"))
        psc = ctxB.enter_context(tc.tile_pool(name="psc", bufs=4, space="PSUM"))

        units = [(0, 0), (1, 0), (0, 1), (1, 1)]
        for (c, half) in units:
            pso_t = [pso.tile([HD + 1, 1024], f32, name=f"pso{c}{half}_{h2}",
                              tag="pso") for h2 in range(2)]
            for m in range(M2):
                psc_t = {}
                for nn in range(2):
                    n_abs = half * 2 + nn
                    nsl = slice(n_abs * 512, (n_abs + 1) * 512)
                    for h2 in range(2):
                        d0 = h2 * 64
                        pt = psc.tile([P, 512], f32,
                                      name=f"psc{c}{half}_{m}_{nn}_{h2}",
                                      tag="psc")
                        psc_t[(nn, h2)] = pt
                        nc.tensor.matmul(
                            pt[:],
                            kT_sb[d0:d0 + 64, c, m * P:(m + 1) * P],
                            qT_sb[d0:d0 + 64, c, nsl],
                            start=True, stop=True)
                ets = {}
                for nn in range(2):
                    for h2 in range(2):
                        et = expp.tile([P, 512], bf16,
                                       name=f"et{c}{half}_{m}_{nn}_{h2}",
                                       tag="expp")
                        ets[(nn, h2)] = et
                        nc.scalar.activation(out=et[:], in_=psc_t[(nn, h2)][:],
                                             func=AF.Exp, scale=SCALE)
                for nn in range(2):
                    for h2 in range(2):
                        nc.tensor.matmul(
                            pso_t[h2][:, nn * 512:(nn + 1) * 512],
                            v_sb[:, m, c * 2 + h2, :],
                            ets[(nn, h2)][:],
                            start=(m == 0), stop=(m == M2 - 1))
            # normalize this unit (copies first so the pso bufs free ASAP)
            osbs = []
            for h2 in range(2):
                osb = osbp.tile([HD + 1, 1024], f32, name=f"osb{c}{half}_{h2}",
                                tag="osb")
                osbs.append(osb)
                nc.vector.tensor_copy(osb[:], pso_t[h2][:])
            for h2 in range(2):
                osb = osbs[h2]
                dcol = att.tile([8, P], f32, name=f"dc{c}{half}_{h2}", tag="dc")
                nc.gpsimd.dma_start(out=dcol[:], in_=osb[HD:HD + 1, :])
                recb = att.tile([8, P], bf16, name=f"rb{c}{half}_{h2}",
                                tag="rb")
                rec = att.tile([8, P], f32, name=f"rc{c}{half}_{h2}", tag="rc")
                nc.vector.reciprocal(rec[:], dcol[:])
                nc.vector.tensor_copy(recb[:], rec[:])
                for nn in range(2):
                    rt = att.tile([1, 512], bf16, name=f"rt{c}{half}_{h2}{nn}",
                                  tag="rt", bufs=4)
                    nc.gpsimd.dma_start(out=rt[:], in_=recb[nn * 4:nn * 4 + 4, :])
                    prb = psc.tile([HD, 512], f32,
                                   name=f"pr{c}{half}_{h2}{nn}", tag="psc")
                    nc.tensor.matmul(prb[:], ones64[:], rt[:],
                                     start=True, stop=True)
                    sl = slice(half * 1024 + nn * 512,
                               half * 1024 + (nn + 1) * 512)
                    osl = slice(nn * 512, (nn + 1) * 512)
                    if h2 == 0:
                        nc.vector.tensor_mul(onorm[0:HD, c, sl], prb[:],
                                             osb[0:HD, osl])
                    else:
                        onm = att.tile([HD, 512], bf16,
                                       name=f"om{c}{half}_{nn}", tag="om")
                        nc.vector.tensor_mul(onm[:], prb[:], osb[0:HD, osl])
                        nc.scalar.dma_start(out=onorm[HD:P, c, sl], in_=onm[:])

        # ---- output projection ----
        ctxB.close()
        pout = ctx.enter_context(tc.tile_pool(name="pout", bufs=4,
                                              space="PSUM"))
        for mo in range(KC):
            for n in range(NQ):
                sl = slice(n * 512, (n + 1) * 512)
                po = pout.tile([P, 512], f32, name=f"po{mo}_{n}", tag="pout")
                for c in range(MC):
                    nc.tensor.matmul(po[:], owt_sb[:, c, mo * P:(mo + 1) * P],
                                     onorm[:, c, sl],
                                     start=(c == 0), stop=(c == MC - 1))
                ost = xs.tile([P, 512], bf16, name=f"ost{mo}_{n}", tag="ost")
                if (mo * NQ + n) % 2 == 0:
                    nc.vector.tensor_copy(ost[:], po[:])
                else:
                    nc.scalar.activation(out=ost[:], in_=po[:], func=AF.Copy)
                nc.sync.dma_start(out=outT[mo * P:(mo + 1) * P, sl], in_=ost[:])

    _legalize_waits(nc, mybir, limit=1)
    return nc


def get_nc():
    if "nc" not in _NC_CACHE:
        _NC_CACHE["nc"] = _build_nc()
    return _NC_CACHE["nc"]


def make_in_maps(x, y, q_w, q_b, kv_w, kv_b, qn_w, qn_b, kn_w, kn_b, out_w, out_b):
    import ml_dtypes
    bf = ml_dtypes.bfloat16
    perm = np.concatenate([np.arange(0, HD, 2), np.arange(1, HD, 2)])
    inv_freq = (1.0 / (10000.0 ** (np.arange(0, HD, 2, dtype=np.float32)
                                   / np.float32(HD)))).astype(np.float32)
    ang = np.arange(S1, dtype=np.float32)[None, :] * inv_freq[:, None]
    cos = np.cos(ang).astype(np.float32)           # (32, S1)
    sin = np.sin(ang).astype(np.float32)
    cosf = np.tile(cos, (4, 1)).astype(bf)
    sinf = np.concatenate([-sin, sin, -sin, sin]).astype(bf)
    # stats selector with 1/HD folded in
    sel = np.zeros((CPC, HPC), np.float32)
    for h in range(HPC):
        sel[h * HD:(h + 1) * HD, h] = 1.0 / HD
    sel_c = np.ascontiguousarray(
        sel.reshape(MC, P, HPC).transpose(1, 0, 2)).astype(bf)  # (P, MC, HPC)

    # broadcast stationary rows: selector rows scaled by LN weight, replicated
    # at partition bases 32n for row-tiled broadcasts
    def rsel_w(w_perm):
        w_full = np.tile(w_perm, HPC)                      # (CPC,)
        r = np.zeros((HPC, CPC), np.float32)
        for h in range(HPC):
            r[h, h * HD:(h + 1) * HD] = w_full[h * HD:(h + 1) * HD]
        return r

    rq = rsel_w(qn_w[perm])
    rk = rsel_w(kn_w[perm])
    rwq = np.zeros((100, CPC), np.float32)
    rwk = np.zeros((36, CPC), np.float32)
    for n in range(NQ):
        rwq[32 * n:32 * n + 4] = rq
    for n in range(NK):
        rwk[32 * n:32 * n + 4] = rk
    rwq = rwq.astype(bf)
    rwk = rwk.astype(bf)

    def chunk_w(wt):                       # (CIN, CPC) -> (P, KC, CPC)
        return np.ascontiguousarray(
            wt.reshape(KC, P, CPC).transpose(1, 0, 2)).astype(bf)

    def perpart(v):                        # (CPC,) -> (P, MC)
        return np.ascontiguousarray(
            v.reshape(MC, P).T).astype(np.float32)

    in_maps = []
    for core in range(8):
        b, g = divmod(core, 4)
        heads = [HPC * g + i for i in range(HPC)]
        qrows = np.concatenate([h * HD + perm for h in heads])
        vrows = np.concatenate([CIN + h * HD + np.arange(HD) for h in heads])
        ocols = np.concatenate([h * HD + np.arange(HD) for h in heads])
        yT = np.ascontiguousarray(y[b].T)              # (CIN, S2)
        in_maps.append({
            "xT": np.ascontiguousarray(x[b].T).astype(bf),
            "yTc": np.ascontiguousarray(
                yT.reshape(KC, P, S2).transpose(1, 0, 2)).astype(bf),
            "qwc": chunk_w(np.ascontiguousarray(q_w[qrows].T)),
            "kwc": chunk_w(np.ascontiguousarray(kv_w[qrows].T)),
            "vwc": chunk_w(np.ascontiguousarray(kv_w[vrows].T)),
            "owc": np.ascontiguousarray(
                out_w[:, ocols].T.reshape(MC, P, CIN).transpose(1, 0, 2)
            ).astype(bf),
            "qb": perpart(q_b[qrows]),
            "kb": perpart(kv_b[qrows]),
            "vb": np.ascontiguousarray(kv_b[vrows]).astype(np.float32),
            "qnb": perpart(np.tile(qn_b[perm], HPC)),
            "knb": perpart(np.tile(kn_b[perm], HPC)),
            "rwq": rwq, "rwk": rwk,
            "cosf": cosf, "sinf": sinf, "sel": sel_c,
        })
    return in_maps


def assemble(parts, out_b):
    result = np.empty((B, S1, CIN), np.float32)
    for b in range(B):
        acc = parts[b * 4].astype(np.float32)
        for g in range(1, 4):
            acc = acc + parts[b * 4 + g].astype(np.float32)
        result[b] = acc.T + out_b[None, :].astype(np.float32)
    return result


def kernel(**inputs):
    args = {k: np.asarray(inputs[k], np.float32) for k in
            ("x", "y", "q_w", "q_b", "kv_w", "kv_b", "qn_w", "qn_b",
             "kn_w", "kn_b", "out_w", "out_b")}
    in_maps = make_in_maps(
        args["x"], args["y"], args["q_w"], args["q_b"], args["kv_w"],
        args["kv_b"], args["qn_w"], args["qn_b"], args["kn_w"], args["kn_b"],
        args["out_w"], args["out_b"])
    from concourse.bass_utils import run_bass_kernel_spmd
    nc = get_nc()
    res = run_bass_kernel_spmd(nc, in_maps, core_ids=list(range(8)))
    parts = [r["outT"] for r in res.results]
    return assemble(parts, args["out_b"])


# revision 25
# speedup vs baseline: 1.0636x; 1.0636x over previous
"""Trainium2 Bass kernel for nn_CrossAttention (b=2, s1=2048, s2=1024, H=16, hd=64).

Sharding: 8 cores = 2 batches x 4 head-groups (4 heads each).

v2 design (vs v1 baseline at ~266us):
  - Dense upfront preamble keeps PE p-state at max: kproj -> LN-k -> qproj ->
    vproj -> LN-q -> RoPE, then an ACT-paced attention phase, then out-proj.
  - LN stats batched: per-slice [4,512] stat matmuls land at PSUM partition
    bases 0/32/64/96 via col tile_position, so the mean/var chain and the
    single ACT Rsqrt cover all slices in one [100,512] pass.
  - LN affine folded into the broadcast matmul stationary (selector rows
    pre-multiplied by qn_w/kn_w on host); apply = 2 DVE ops reading PSUM.
  - Scores for the head pair of each 128-channel chunk are emitted
    back-to-back with K=64 row tiles (0,0)/(64,0) -> concurrent on the PE.
  - Softmax denominator rides as a ones-column of v (M=65 AV matmuls).
  - Per-unit (pair x s1-half) normalization via DMA-replicated reciprocal
    rows (no PSUM, no PE).
  - Output written bf16 (host accumulates partials in fp32).
"""

import numpy as np

B, S1, S2, CIN, H, HD = 2, 2048, 1024, 1024, 16, 64
HPC = 4                # heads per core
CPC = HPC * HD         # 256 channels per core
P = 128
KC = CIN // P          # 8 cin chunks
MC = CPC // P          # 2 channel chunks (= head pairs)
NQ = S1 // 512         # 4 s1 slices
NK = S2 // 512         # 2 s2 slices
M2 = S2 // P           # 8 s2 chunks
SCALE = HD ** -0.5
EPS = 1e-6

_NC_CACHE = {}


def _legalize_waits(nc, mybir, limit=1):
    """Split instructions carrying >limit semaphore waits into a chain of
    single-wait NOPs on the same engine followed by the instruction."""
    n_split = 0
    for fn in nc.m.functions:
        for bb in fn.blocks:
            out = []
            for inst in bb.instructions:
                si = inst.sync_info
                waits = list(si.on_wait) if si is not None and si.on_wait else []
                if len(waits) > limit:
                    for i, w in enumerate(waits[:-limit]):
                        nop = mybir.InstNoOp(
                            name=f"{inst.name}-lw{i}", ins=[], outs=[])
                        nop.engine = inst.engine
                        nop.sync_info = mybir.SyncInfo(on_wait=[w], on_update=[])
                        try:
                            nc.register_instruction(nop, overwrite=True)
                        except Exception:
                            pass
                        out.append(nop)
                    inst.sync_info = mybir.SyncInfo(
                        on_wait=waits[-limit:], on_update=list(si.on_update))
                    n_split += 1
                out.append(inst)
            bb.instructions = out
    return n_split


def _build_nc():
    from contextlib import ExitStack

    import concourse.bass as bass
    import concourse.mybir as mybir
    import concourse.tile as tile

    f32 = mybir.dt.float32
    bf16 = mybir.dt.bfloat16
    AF = mybir.ActivationFunctionType
    OP = mybir.AluOpType

    nc = bass.Bass()

    def din(name, shape, dt=bf16):
        return nc.dram_tensor(name, list(shape), dt, kind="ExternalInput")

    xT = din("xT", (CIN, S1))
    yTc = din("yTc", (P, KC, S2))
    qwc = din("qwc", (P, KC, CPC))
    kwc = din("kwc", (P, KC, CPC))
    vwc = din("vwc", (P, KC, CPC))
    owc = din("owc", (P, MC, CIN))
    qb = din("qb", (P, MC), f32)
    kb = din("kb", (P, MC), f32)
    vb = din("vb", (CPC,), f32)
    qnb = din("qnb", (P, MC), f32)
    knb = din("knb", (P, MC), f32)
    rwq = din("rwq", (100, CPC))
    rwk = din("rwk", (36, CPC))
    seld = din("sel", (P, MC, HPC))
    cosf = din("cosf", (P, S1))
    sinf = din("sinf", (P, S1))
    outT = nc.dram_tensor("outT", [CIN, S1], bf16, kind="ExternalOutput")

    with tile.TileContext(nc) as tc, ExitStack() as ctx:
        consts = ctx.enter_context(tc.tile_pool(name="consts", bufs=1))
        pers = ctx.enter_context(tc.tile_pool(name="pers", bufs=1))
        xs = ctx.enter_context(tc.tile_pool(name="xs", bufs=4))
        tmp = ctx.enter_context(tc.tile_pool(name="tmp", bufs=2))
        rop = ctx.enter_context(tc.tile_pool(name="rop", bufs=2))
        att = ctx.enter_context(tc.tile_pool(name="att", bufs=2))
        expp = ctx.enter_context(tc.tile_pool(name="expp", bufs=6))
        osbp = ctx.enter_context(tc.tile_pool(name="osbp", bufs=4))

        ctxA = ctx.enter_context(ExitStack())
        pp = ctxA.enter_context(tc.tile_pool(name="pp", bufs=4, space="PSUM"))
        pst = ctxA.enter_context(tc.tile_pool(name="pst", bufs=2, space="PSUM"))
        pvv = ctxA.enter_context(tc.tile_pool(name="pvv", bufs=2, space="PSUM"))

        # ---- constants ----
        # k-proj inputs first (scalar ring), split per k-chunk for streaming
        kwt_sb = consts.tile([P, KC, CPC], bf16)
        yT_sb = consts.tile([P, KC, S2], bf16)
        for k in range(KC):
            nc.scalar.dma_start(out=kwt_sb[:, k, :], in_=kwc[:, k, :])
            nc.scalar.dma_start(out=yT_sb[:, k, :], in_=yTc[:, k, :])
        # q-proj weights on sync ring (x stream follows there)
        qwt_sb = consts.tile([P, KC, CPC], bf16)
        for k in range(KC):
            nc.sync.dma_start(out=qwt_sb[:, k, :], in_=qwc[:, k, :])
        # the rest on gpsimd/vector rings, ordered by first use
        sel_sb = consts.tile([P, MC, HPC], bf16)
        nc.gpsimd.dma_start(out=sel_sb, in_=seld[:])
        rwk_sb = consts.tile([36, CPC], bf16)
        nc.gpsimd.dma_start(out=rwk_sb, in_=rwk[:])
        rwq_sb = consts.tile([100, CPC], bf16)
        nc.gpsimd.dma_start(out=rwq_sb, in_=rwq[:])
        kb_sb = consts.tile([P, MC], f32)
        nc.gpsimd.dma_start(out=kb_sb, in_=kb[:])
        qb_sb = consts.tile([P, MC], f32)
        nc.gpsimd.dma_start(out=qb_sb, in_=qb[:])
        knb_sb = consts.tile([P, MC], f32)
        nc.gpsimd.dma_start(out=knb_sb, in_=knb[:])
        qnb_sb = consts.tile([P, MC], f32)
        nc.gpsimd.dma_start(out=qnb_sb, in_=qnb[:])
        vwt_sb = consts.tile([P, KC, CPC], bf16)
        for k in range(KC):
            nc.scalar.dma_start(out=vwt_sb[:, k, :], in_=vwc[:, k, :])
        cosf_sb = consts.tile([P, S1], bf16)
        nc.scalar.dma_start(out=cosf_sb, in_=cosf[:])
        sinf_sb = consts.tile([P, S1], bf16)
        nc.scalar.dma_start(out=sinf_sb, in_=sinf[:])
        owt_sb = consts.tile([P, MC, CIN], bf16)
        nc.scalar.dma_start(out=owt_sb, in_=owc[:])
        vbb_sb = consts.tile([P, CPC], f32)
        vb_ap = vb[:]
        nc.gpsimd.dma_start(
            out=vbb_sb,
            in_=bass.AP(tensor=vb_ap.tensor, offset=vb_ap.offset,
                        ap=[[0, P]] + list(vb_ap.ap)),
        )
        eps_t = consts.tile([100, 1], f32)
        nc.vector.memset(eps_t, EPS)
        dm = consts.tile([1, 8], f32)
        nc.vector.memset(dm, 1.0)
        dmo = consts.tile([1, 8], f32)
        # warm the ACT sqrt table set early (hidden under DMA warmup)
        nc.scalar.activation(out=dmo[:], in_=dm[:], func=AF.Sqrt)

        # ---- persistent activations ----
        qT_sb = pers.tile([P, MC, S1], bf16)
        kT_sb = pers.tile([P, MC, S2], bf16)
        # v column 64 is all-ones: the AV matmul emits the softmax denominator
        # as PSUM row 64 for free (M=65 is paced by N, same as M=64).
        v_sb = pers.tile([P, M2, HPC, HD + 1], bf16)
        onorm = pers.tile([P, MC, S1], bf16)
        nc.vector.memset(v_sb[:, :, :, HD:HD + 1], 1.0)
        ones64 = consts.tile([1, HD], bf16)
        nc.vector.memset(ones64, 1.0)

        # ---- layernorm helper (batched stats at 32-partition bases) ----
        def layernorm(src, NS, rw_sb, b_sb, pfx):
            """src: [P, MC, NS*512]; stats for slice n at partition base 32n."""
            nparts = 32 * (NS - 1) + 4
            pss = pst.tile([nparts, 512], f32, name=f"{pfx}pss", tag="st")
            psq = pst.tile([nparts, 512], f32, name=f"{pfx}psq", tag="st")
            nc.vector.memset(pss, 0.0)
            nc.vector.memset(psq, 0.0)
            sqs = []
            for n in range(NS):
                sl = slice(n * 512, (n + 1) * 512)
                for c in range(MC):
                    sq = tmp.tile([P, 512], bf16, name=f"{pfx}sq{c}_{n}",
                                  tag="sq", bufs=4)
                    nc.vector.tensor_mul(sq[:], src[:, c, sl], src[:, c, sl])
                    sqs.append(sq)
            for c in range(MC):
                st = (c == 0)
                sp = (c == MC - 1)
                for n in range(NS):
                    sl = slice(n * 512, (n + 1) * 512)
                    r0 = 32 * n
                    nc.tensor.matmul(pss[r0:r0 + 4, :], sel_sb[:, c, :],
                                     src[:, c, sl], start=st, stop=sp,
                                     tile_position=(0, r0),
                                     skip_group_check=True)
                    nc.tensor.matmul(psq[r0:r0 + 4, :], sel_sb[:, c, :],
                                     sqs[n * MC + c][:], start=st, stop=sp,
                                     tile_position=(0, r0),
                                     skip_group_check=True)
            # mean/var chain over the whole batched span (junk rows harmless)
            mu = tmp.tile([nparts, 512], f32, name=f"{pfx}mu", tag="mu")
            nc.vector.tensor_copy(mu[:], pss[:])
            musq = tmp.tile([nparts, 512], f32, name=f"{pfx}musq", tag="musq")
            nc.vector.tensor_mul(musq[:], mu[:], mu[:])
            var = tmp.tile([nparts, 512], f32, name=f"{pfx}var", tag="var")
            nc.vector.tensor_sub(var[:], psq[:], musq[:])
            sd = tmp.tile([nparts, 512], f32, name=f"{pfx}sd", tag="sd")
            nc.scalar.activation(out=sd[:], in_=var[:], func=AF.Sqrt,
                                 bias=eps_t[0:nparts, :], scale=1.0)
            Af = tmp.tile([nparts, 512], f32, name=f"{pfx}Af", tag="Af")
            nc.vector.reciprocal(Af[:], sd[:])
            Ab = tmp.tile([nparts, 512], bf16, name=f"{pfx}Ab", tag="Ab")
            nc.vector.tensor_copy(Ab[:], Af[:])
            Bb = tmp.tile([nparts, 512], bf16, name=f"{pfx}Bb", tag="Bb")
            nc.vector.scalar_tensor_tensor(
                out=Bb[:], in0=mu[:], scalar=-1.0, in1=Af[:],
                op0=OP.mult, op1=OP.mult)                     # B = -mu*rstd
            # broadcast (w folded into stationary rows) + apply
            for c in range(MC):
                for n in range(NS):
                    r0 = 32 * n
                    sl = slice(n * 512, (n + 1) * 512)
                    psA = pp.tile([P, 512], f32, name=f"{pfx}psA{c}_{n}",
                                  tag="proj")
                    nc.tensor.matmul(
                        psA[:], rw_sb[r0:r0 + 4, c * P:(c + 1) * P],
                        Ab[r0:r0 + 4, :], start=True, stop=True,
                        tile_position=(r0, 0))
                    psB = pp.tile([P, 512], f32, name=f"{pfx}psB{c}_{n}",
                                  tag="proj")
                    nc.tensor.matmul(
                        psB[:], rw_sb[r0:r0 + 4, c * P:(c + 1) * P],
                        Bb[r0:r0 + 4, :], start=True, stop=True,
                        tile_position=(r0, 0))
                    nc.vector.tensor_mul(src[:, c, sl], psA[:],
                                         src[:, c, sl])
                    nc.vector.scalar_tensor_tensor(
                        out=src[:, c, sl], in0=psB[:],
                        scalar=b_sb[:, c:c + 1], in1=src[:, c, sl],
                        op0=OP.add, op1=OP.add)

        # ---- k projection ----
        for n2 in range(NK):
            sl = slice(n2 * 512, (n2 + 1) * 512)
            ps = [pp.tile([P, 512], f32, name=f"psk{c}_{n2}", tag="proj")
                  for c in range(MC)]
            for k in range(KC):
                for c in range(MC):
                    nc.tensor.matmul(
                        ps[c][:], kwt_sb[:, k, c * P:(c + 1) * P],
                        yT_sb[:, k, sl], start=(k == 0), stop=(k == KC - 1))
            for c in range(MC):
                nc.vector.tensor_scalar_add(kT_sb[:, c, sl], ps[c][:],
                                            kb_sb[:, c:c + 1])

        # ---- q projection (x streamed per k-chunk, 1024-col halves) ----
        def qproj_half(half):
            hsl = slice(half * 1024, (half + 1) * 1024)
            ps = [[pp.tile([P, 512], f32, name=f"psq{c}_{half}{n}", tag="proj")
                   for n in range(2)] for c in range(MC)]
            for k in range(KC):
                xt = xs.tile([P, 1024], bf16, name=f"xt{half}_{k}", tag="xs")
                nc.sync.dma_start(out=xt, in_=xT[k * P:(k + 1) * P, hsl])
                for c in range(MC):
                    for n in range(2):
                        nc.tensor.matmul(
                            ps[c][n][:], qwt_sb[:, k, c * P:(c + 1) * P],
                            xt[:, n * 512:(n + 1) * 512],
                            start=(k == 0), stop=(k == KC - 1))
            for c in range(MC):
                for n in range(2):
                    sl = slice(half * 1024 + n * 512, half * 1024 + (n + 1) * 512)
                    nc.vector.tensor_scalar_add(qT_sb[:, c, sl], ps[c][n][:],
                                                qb_sb[:, c:c + 1])

        # ---- v projection chunk: v[s2-part, channel] ----
        def vproj(m):
            psv = pvv.tile([P, CPC], f32, name=f"psv{m}", tag="pv")
            for k in range(KC):
                nc.tensor.matmul(
                    psv[:], yT_sb[:, k, m * P:(m + 1) * P], vwt_sb[:, k, :],
                    start=(k == 0), stop=(k == KC - 1))
            nc.vector.tensor_add(
                v_sb[:, m, :, 0:HD],
                psv.rearrange("p (h d) -> p h d", h=HPC),
                vbb_sb.rearrange("p (h d) -> p h d", h=HPC))

        # fill pool-ring boundary bubbles: alternate independent PE work
        qproj_half(0)
        for m in range(4):
            vproj(m)
        layernorm(kT_sb, NK, rwk_sb, knb_sb, "k")
        qproj_half(1)
        for m in range(4, M2):
            vproj(m)
        layernorm(qT_sb, NQ, rwq_sb, qnb_sb, "q")

        # warm the ACT exp table set (hidden under RoPE / attention warmup)
        nc.scalar.activation(out=dmo[:], in_=dm[:], func=AF.Exp)

        # ---- RoPE on q, in place (after LN, like the reference) ----
        for c in range(MC):
            for half in range(2):
                hsl = slice(half * 1024, (half + 1) * 1024)
                qsw = rop.tile([P, 1024], bf16, name=f"qsw{c}_{half}",
                               tag="qsw")
                for blk in range(4):
                    d_src = (blk ^ 1) * 32
                    nc.scalar.dma_start(
                        out=qsw[blk * 32:(blk + 1) * 32, :],
                        in_=qT_sb[d_src:d_src + 32, c, hsl])
                t = rop.tile([P, 1024], bf16, name=f"rt{c}_{half}", tag="rt")
                nc.vector.tensor_mul(t[:], qsw[:], sinf_sb[:, hsl])
                nc.vector.tensor_mul(qT_sb[:, c, hsl], qT_sb[:, c, hsl],
                                     cosf_sb[:, hsl])
                nc.vector.tensor_add(qT_sb[:, c, hsl], qT_sb[:, c, hsl], t[:])

        # ---- attention: units = (pair c, s1-half), ACT-paced ----
        ctxA.close()
        ctxB = ctx.enter_context(ExitStack())
        pso = ctxB.enter_context(tc.tile_pool(name="pso", bufs=2, space="PSUM"))
        psc = ctxB.enter_context(tc.tile_pool(name="psc", bufs=4, space="PSUM"))

        def emit_norm(c, half, pso_t):
            # normalize a unit (copies first so the pso bufs free ASAP)
            osbs = []
            for h2 in range(2):
                osb = osbp.tile([HD + 1, 1024], f32, name=f"osb{c}{half}_{h2}",
                                tag="osb")
                osbs.append(osb)
                nc.vector.tensor_copy(osb[:], pso_t[h2][:])
            for h2 in range(2):
                osb = osbs[h2]
                dcol = att.tile([8, P], f32, name=f"dc{c}{half}_{h2}", tag="dc")
                nc.gpsimd.dma_start(out=dcol[:], in_=osb[HD:HD + 1, :])
                recb = att.tile([8, P], bf16, name=f"rb{c}{half}_{h2}",
                                tag="rb")
                rec = att.tile([8, P], f32, name=f"rc{c}{half}_{h2}", tag="rc")
                nc.vector.reciprocal(rec[:], dcol[:])
                nc.vector.tensor_copy(recb[:], rec[:])
                for nn in range(2):
                    rt = att.tile([1, 512], bf16, name=f"rt{c}{half}_{h2}{nn}",
                                  tag="rt", bufs=4)
                    nc.gpsimd.dma_start(out=rt[:], in_=recb[nn * 4:nn * 4 + 4, :])
                    prb = psc.tile([HD, 512], f32,
                                   name=f"pr{c}{half}_{h2}{nn}", tag="psc")
                    nc.tensor.matmul(prb[:], ones64[:], rt[:],
                                     start=True, stop=True)
                    sl = slice(half * 1024 + nn * 512,
                               half * 1024 + (nn + 1) * 512)
                    osl = slice(nn * 512, (nn + 1) * 512)
                    if h2 == 0:
                        nc.vector.tensor_mul(onorm[0:HD, c, sl], prb[:],
                                             osb[0:HD, osl])
                    else:
                        onm = att.tile([HD, 512], bf16,
                                       name=f"om{c}{half}_{nn}", tag="om")
                        nc.vector.tensor_mul(onm[:], prb[:], osb[0:HD, osl])
                        nc.scalar.dma_start(out=onorm[HD:P, c, sl], in_=onm[:])

        # ---- output projection ----
        # emit each unit's m-loop; defer its normalize until after the NEXT
        # unit's m-loop so the serial normalize chain never blocks the PE
        # queue head (which starved ACT and re-ramped the PE p-state).
        units = [(0, 0), (1, 0), (0, 1), (1, 1)]
        pending = None
        for (c, half) in units:
            pso_t = [pso.tile([HD + 1, 1024], f32, name=f"pso{c}{half}_{h2}",
                              tag="pso") for h2 in range(2)]
            for m in range(M2):
                psc_t = {}
                for nn in range(2):
                    n_abs = half * 2 + nn
                    nsl = slice(n_abs * 512, (n_abs + 1) * 512)
                    for h2 in range(2):
                        d0 = h2 * 64
                        pt = psc.tile([P, 512], f32,
                                      name=f"psc{c}{half}_{m}_{nn}_{h2}",
                                      tag="psc")
                        psc_t[(nn, h2)] = pt
                        nc.tensor.matmul(
                            pt[:],
                            kT_sb[d0:d0 + 64, c, m * P:(m + 1) * P],
                            qT_sb[d0:d0 + 64, c, nsl],
                            start=True, stop=True)
                ets = {}
                for nn in range(2):
                    for h2 in range(2):
                        et = expp.tile([P, 512], bf16,
                                       name=f"et{c}{half}_{m}_{nn}_{h2}",
                                       tag="expp")
                        ets[(nn, h2)] = et
                        nc.scalar.activation(out=et[:], in_=psc_t[(nn, h2)][:],
                                             func=AF.Exp, scale=SCALE)
                for nn in range(2):
                    for h2 in range(2):
                        nc.tensor.matmul(
                            pso_t[h2][:, nn * 512:(nn + 1) * 512],
                            v_sb[:, m, c * 2 + h2, :],
                            ets[(nn, h2)][:],
                            start=(m == 0), stop=(m == M2 - 1))
            if pending is not None:
                emit_norm(*pending)
            pending = (c, half, pso_t)
        emit_norm(*pending)
        ctxB.close()
        pout = ctx.enter_context(tc.tile_pool(name="pout", bufs=4,
                                              space="PSUM"))
        for mo in range(KC):
            for n in range(NQ):
                sl = slice(n * 512, (n + 1) * 512)
                po = pout.tile([P, 512], f32, name=f"po{mo}_{n}", tag="pout")
                for c in range(MC):
                    nc.tensor.matmul(po[:], owt_sb[:, c, mo * P:(mo + 1) * P],
                                     onorm[:, c, sl],
                                     start=(c == 0), stop=(c == MC - 1))
                ost = xs.tile([P, 512], bf16, name=f"ost{mo}_{n}", tag="ost")
                if (mo * NQ + n) % 2 == 0:
                    nc.vector.tensor_copy(ost[:], po[:])
                else:
                    nc.scalar.activation(out=ost[:], in_=po[:], func=AF.Copy)
                nc.sync.dma_start(out=outT[mo * P:(mo + 1) * P, sl], in_=ost[:])

    _legalize_waits(nc, mybir, limit=1)
    return nc


def get_nc():
    if "nc" not in _NC_CACHE:
        _NC_CACHE["nc"] = _build_nc()
    return _NC_CACHE["nc"]


def make_in_maps(x, y, q_w, q_b, kv_w, kv_b, qn_w, qn_b, kn_w, kn_b, out_w, out_b):
    import ml_dtypes
    bf = ml_dtypes.bfloat16
    perm = np.concatenate([np.arange(0, HD, 2), np.arange(1, HD, 2)])
    inv_freq = (1.0 / (10000.0 ** (np.arange(0, HD, 2, dtype=np.float32)
                                   / np.float32(HD)))).astype(np.float32)
    ang = np.arange(S1, dtype=np.float32)[None, :] * inv_freq[:, None]
    cos = np.cos(ang).astype(np.float32)           # (32, S1)
    sin = np.sin(ang).astype(np.float32)
    cosf = np.tile(cos, (4, 1)).astype(bf)
    sinf = np.concatenate([-sin, sin, -sin, sin]).astype(bf)
    # stats selector with 1/HD folded in
    sel = np.zeros((CPC, HPC), np.float32)
    for h in range(HPC):
        sel[h * HD:(h + 1) * HD, h] = 1.0 / HD
    sel_c = np.ascontiguousarray(
        sel.reshape(MC, P, HPC).transpose(1, 0, 2)).astype(bf)  # (P, MC, HPC)

    # broadcast stationary rows: selector rows scaled by LN weight, replicated
    # at partition bases 32n for row-tiled broadcasts
    def rsel_w(w_perm):
        w_full = np.tile(w_perm, HPC)                      # (CPC,)
        r = np.zeros((HPC, CPC), np.float32)
        for h in range(HPC):
            r[h, h * HD:(h + 1) * HD] = w_full[h * HD:(h + 1) * HD]
        return r

    rq = rsel_w(qn_w[perm])
    rk = rsel_w(kn_w[perm])
    rwq = np.zeros((100, CPC), np.float32)
    rwk = np.zeros((36, CPC), np.float32)
    for n in range(NQ):
        rwq[32 * n:32 * n + 4] = rq
    for n in range(NK):
        rwk[32 * n:32 * n + 4] = rk
    rwq = rwq.astype(bf)
    rwk = rwk.astype(bf)

    def chunk_w(wt):                       # (CIN, CPC) -> (P, KC, CPC)
        return np.ascontiguousarray(
            wt.reshape(KC, P, CPC).transpose(1, 0, 2)).astype(bf)

    def perpart(v):                        # (CPC,) -> (P, MC)
        return np.ascontiguousarray(
            v.reshape(MC, P).T).astype(np.float32)

    in_maps = []
    for core in range(8):
        b, g = divmod(core, 4)
        heads = [HPC * g + i for i in range(HPC)]
        qrows = np.concatenate([h * HD + perm for h in heads])
        vrows = np.concatenate([CIN + h * HD + np.arange(HD) for h in heads])
        ocols = np.concatenate([h * HD + np.arange(HD) for h in heads])
        yT = np.ascontiguousarray(y[b].T)              # (CIN, S2)
        in_maps.append({
            "xT": np.ascontiguousarray(x[b].T).astype(bf),
            "yTc": np.ascontiguousarray(
                yT.reshape(KC, P, S2).transpose(1, 0, 2)).astype(bf),
            "qwc": chunk_w(np.ascontiguousarray(q_w[qrows].T)),
            "kwc": chunk_w(np.ascontiguousarray(kv_w[qrows].T)),
            "vwc": chunk_w(np.ascontiguousarray(kv_w[vrows].T)),
            "owc": np.ascontiguousarray(
                out_w[:, ocols].T.reshape(MC, P, CIN).transpose(1, 0, 2)
            ).astype(bf),
            "qb": perpart(q_b[qrows]),
            "kb": perpart(kv_b[qrows]),
            "vb": np.ascontiguousarray(kv_b[vrows]).astype(np.float32),
            "qnb": perpart(np.tile(qn_b[perm], HPC)),
            "knb": perpart(np.tile(kn_b[perm], HPC)),
            "rwq": rwq, "rwk": rwk,
            "cosf": cosf, "sinf": sinf, "sel": sel_c,
        })
    return in_maps


def assemble(parts, out_b):
    result = np.empty((B, S1, CIN), np.float32)
    for b in range(B):
        acc = parts[b * 4].astype(np.float32)
        for g in range(1, 4):
            acc = acc + parts[b * 4 + g].astype(np.float32)
        result[b] = acc.T + out_b[None, :].astype(np.float32)
    return result


def kernel(**inputs):
    args = {k: np.asarray(inputs[k], np.float32) for k in
            ("x", "y", "q_w", "q_b", "kv_w", "kv_b", "qn_w", "qn_b",
             "kn_w", "kn_b", "out_w", "out_b")}
    in_maps = make_in_maps(
        args["x"], args["y"], args["q_w"], args["q_b"], args["kv_w"],
        args["kv_b"], args["qn_w"], args["qn_b"], args["kn_w"], args["kn_b"],
        args["out_w"], args["out_b"])
    from concourse.bass_utils import run_bass_kernel_spmd
    nc = get_nc()
    res = run_bass_kernel_spmd(nc, in_maps, core_ids=list(range(8)))
    parts = [r["outT"] for r in res.results]
    return assemble(parts, args["out_b"])
